# revision 12
# baseline (speedup 1.0000x reference)
# kernel.py — Bidirectional masked-GRU-with-predictor on 8 Trainium2 NeuronCores.
#
# Problem (reference.py): B=128, T=1024, H=512
#   per step, per direction:
#     x_in = where(mask, predictor(h), x)            predictor: Linear(H,H)->ReLU->Linear(H,1)->Tanh
#     h    = GRUCell(h, x_in)                        PyTorch gate order (r, z, n)
#   output [B, T, 2H] = concat(fwd hidden states, time-reversed bwd hidden states)
#
# Sharding: 8 cores = 2 directions x 4 batch groups of 32.  All cores run the
# SAME Bass program; per-core data differs (bwd cores get time-reversed x/mask
# and their outputs are flipped back on the host).
#
# On-core layout ("feature-major, chunk-in-free"):
#   h^T kept as [128 partitions = feature%128, (j,b)] where j = feature//128 (4 chunks),
#   b = local batch (32).  Big matmul: stationary = W^T 128x128 blocks (fp16, FWL),
#   moving = h chunks; gates + predictor-hidden land in PSUM feature-major, so the
#   new h is produced directly in the layout the next step's matmul consumes.
#
# v3 changes vs baseline:
#   - Each PSUM gate region is opened by ONE fast padded bias matmul
#     (K=128 stationary shared across regions, selector moving operand) in
#     place of the baseline's slow K=4 bias matmuls (~270ns/pair each).
#     NOTE: start=True clears has_written for the whole tile, so a region
#     must be opened by a single full-width matmul — per-j starts lose the
#     sibling columns' accumulation (overwritten by the later gi matmul).
#   - GIN's bias rides as a 3rd row of its rank-3 gi matmul (ones row in S2).
#   - PSUM operands of the elementwise chain are staged to fp16 SBUF off the
#     critical path so the chain runs in DVE 2x mode (165ns vs 345ns/op).
#   - (1-z) computed directly as sigmoid(-G_z) via the ACT scale parameter;
#     h' = h + (1-z)*(n - h), so z itself is never materialized.
#   - PE issue order arranged so the predictor chain overlaps W streaming;
#     each PSUM tile gets its own bank (tiles padded to 512 f32 cols).

import numpy as np

B, T, H = 128, 1024, 512
NCORES = 8
BL = B // 4          # 32: batch per core (4 groups x 2 directions)
KC = H // 128        # 4 contraction chunks
MC = (3 * H + H) // 128  # 16 stationary m-chunks (w_hh 12 + p_w1 4)
U_DEF = 32           # time steps per For_i iteration

_cache = {}


def _build_program(t_steps=T, u_steps=U_DEF, bl=BL, n_cores=NCORES):
    import concourse.bacc as bacc
    import concourse.bass as bass
    import concourse.tile as tile
    from concourse.tile import add_dep_helper
    from concourse import mybir

    f16 = mybir.dt.float16
    f32 = mybir.dt.float32

    nc = bacc.Bacc(
        "TRN2",
        target_bir_lowering=False,
        debug=False,
        enable_asserts=False,
        num_devices=n_cores,
    )

    # ---- DRAM tensors (per-core data; same names on every core) ----
    d_wt = nc.dram_tensor("wt", [128, MC * KC * 128], f16, kind="ExternalInput").ap()
    d_gi = nc.dram_tensor("gilhs", [3, 12 * 128], f16, kind="ExternalInput").ap()
    d_bp = nc.dram_tensor("bpad", [128, 128], f16, kind="ExternalInput").ap()
    d_es = nc.dram_tensor("esel", [128, 4 * 128], f16, kind="ExternalInput").ap()
    d_pw2 = nc.dram_tensor("pw2t", [128, KC], f16, kind="ExternalInput").ap()
    d_pb2h = nc.dram_tensor("pb2h", [1, 1], f16, kind="ExternalInput").ap()
    d_mb = nc.dram_tensor("m_bcast", [128, t_steps * 4 * bl], f16, kind="ExternalInput").ap()
    d_a = nc.dram_tensor("a_arr", [t_steps, bl], f16, kind="ExternalInput").ap()
    d_one = nc.dram_tensor("ones_row", [1, u_steps * bl], f16, kind="ExternalInput").ap()
    d_m = nc.dram_tensor("m_arr", [t_steps, bl], f16, kind="ExternalInput").ap()
    d_out = nc.dram_tensor(
        "outl", [t_steps, 128, KC, bl], f16, kind="ExternalOutput"
    ).ap()

    Tanh = mybir.ActivationFunctionType.Tanh
    Sigmoid = mybir.ActivationFunctionType.Sigmoid
    R = KC * bl  # 128: one gate region's free size

    with tile.TileContext(nc) as tc:
        import contextlib

        with contextlib.ExitStack() as ctx:
            consts = ctx.enter_context(tc.tile_pool(name="consts", bufs=1))
            psum = ctx.enter_context(tc.tile_pool(name="psum", bufs=1, space="PSUM"))
            work = ctx.enter_context(tc.tile_pool(name="work", bufs=2))
            io = ctx.enter_context(tc.tile_pool(name="io", bufs=2))

            # ---- constant preload ----
            WT = consts.tile([128, MC * KC * 128], f16, tag="WT")
            GIL = consts.tile([3, 12 * 128], f16, tag="GIL")
            BP = consts.tile([128, 128], f16, tag="BP")
            ES = consts.tile([128, 4 * 128], f16, tag="ES")
            PW2 = consts.tile([128, KC], f16, tag="PW2")
            PB2H = consts.tile([1, 1], f16, tag="PB2H")
            for dst, src in (
                (WT, d_wt), (GIL, d_gi), (BP, d_bp),
                (ES, d_es), (PW2, d_pw2), (PB2H, d_pb2h),
            ):
                nc.sync.dma_start(out=dst, in_=src)

            # persistent ping-pong hidden state, fp16, [128, (j,b)]
            h0 = consts.tile([128, R], f16, tag="h0")
            h1 = consts.tile([128, R], f16, tag="h1")
            nc.vector.memset(h0, 0.0)
            nc.vector.memset(h1, 0.0)
            h_tiles = [h0, h1]
            prologue_done = False

            # persistent PSUM accumulators — one full bank (512 f32 cols)
            # per region so start=True clears cannot touch a sibling region.
            GNB = psum.tile([128, 512], f32, tag="GNB")
            PHB = psum.tile([128, 512], f32, tag="PHB")
            GRB = psum.tile([128, 512], f32, tag="GRB")
            GZB = psum.tile([128, 512], f32, tag="GZB")
            GIB = psum.tile([128, 512], f32, tag="GIB")
            PRB = psum.tile([128, 512], f32, tag="PRB")
            PNB = psum.tile([128, 512], f32, tag="PNB")
            n_reg = GNB[:, 0:R]
            ph_reg = PHB[:, 0:R]
            r_reg = GRB[:, 0:R]
            z_reg = GZB[:, 0:R]
            GIN = GIB[:, 0:R]
            PRD = PRB[0:1, 0:bl]
            PREN = PNB[:, 0:R]

            def w_block(m, k):
                bi = m * KC + k
                return WT[:, bi * 128:(bi + 1) * 128]

            def pe_order(a_ins, b_ins):
                # force PE issue order: a runs after b (ordering only)
                add_dep_helper(a_ins.ins, b_ins.ins, sync=False)

            def emit_step(u, h_cur, h_new, S2, MB, t_dyn, prev_last):
                gi_rhs = S2[:, u * bl:(u + 1) * bl]
                mb_u = MB[0:1, u * bl:(u + 1) * bl]

                def bias_mm(region, ri, prev):
                    mm = nc.tensor.matmul(
                        region, BP, ES[:, ri * 128:(ri + 1) * 128],
                        start=True, stop=False, skip_group_check=True,
                    )
                    if prev is not None:
                        pe_order(mm, prev)
                    return mm

                def w_mms(region, base_m, prev, stop_k3):
                    last = prev
                    for j in range(KC):
                        for k in range(KC):
                            mm = nc.tensor.matmul(
                                region[:, j * bl:(j + 1) * bl],
                                w_block(base_m + j, k),
                                h_cur[:, k * bl:(k + 1) * bl],
                                start=False, stop=(stop_k3 and k == KC - 1),
                                skip_group_check=True,
                            )
                            pe_order(mm, last)
                            last = mm
                    return last

                def w_mms_jrange(region, base_m, prev, stop_k3, jlo, jhi):
                    last = prev
                    for j in range(jlo, jhi):
                        for k in range(KC):
                            mm = nc.tensor.matmul(
                                region[:, j * bl:(j + 1) * bl],
                                w_block(base_m + j, k),
                                h_cur[:, k * bl:(k + 1) * bl],
                                start=False, stop=(stop_k3 and k == KC - 1),
                                skip_group_check=True,
                            )
                            pe_order(mm, last)
                            last = mm
                    return last

                def gi_mms(region, g_idx, prev, start, stop):
                    last = prev
                    for j in range(KC):
                        gj = g_idx * KC + j
                        mm = nc.tensor.matmul(
                            region[:, j * bl:(j + 1) * bl],
                            GIL[:, gj * 128:(gj + 1) * 128],
                            gi_rhs,
                            start=start, stop=stop, skip_group_check=True,
                        )
                        pe_order(mm, last)
                        last = mm
                    return last

                # --- PE stream ---  (PH for THIS step was computed by the
                # previous step via PH = P.h + P.e; see tail below)
                last = bias_mm(r_reg, 2, prev_last)
                last = w_mms(r_reg, 0, last, False)              # W_r

                # masked relu of PH in one stt: max(PH,0) * m  (m pre-broadcast)
                relu_sb = work.tile([128, R], f16, tag="relu")
                nc.vector.scalar_tensor_tensor(
                    out=relu_sb, in0=ph_reg, scalar=0.0,
                    in1=MBC[:, u * R:(u + 1) * R],
                    op0=mybir.AluOpType.max, op1=mybir.AluOpType.mult,
                )

                # PRD = b2*m + sum_k pw2.relu_m  -> pred*mask = tanh(PRD)
                mm = nc.tensor.matmul(
                    PRD, PB2H, mb_u, start=True, stop=False,
                    skip_group_check=True,
                )
                pe_order(mm, last)
                last = mm
                for k in range(KC):
                    mm = nc.tensor.matmul(
                        PRD, PW2[:, k:k + 1], relu_sb[:, k * bl:(k + 1) * bl],
                        start=False, stop=(k == KC - 1),
                        skip_group_check=True,
                    )
                    pe_order(mm, last)
                    last = mm

                # S2 row0 = mask * tanh(PRD/mask...) == tanh(PRD) since m in {0,1}
                nc.scalar.activation(
                    out=S2[0:1, u * bl:(u + 1) * bl], in_=PRD, func=Tanh,
                )

                last = bias_mm(n_reg, 0, last)
                last = w_mms(n_reg, 8, last, True)               # W_n

                # stage G_n to SBUF fp16 (off critical path)
                gn_sb = work.tile([128, R], f16, tag="gn_sb")
                nc.vector.tensor_copy(gn_sb, n_reg)

                last = gi_mms(r_reg, 0, last, False, True)       # gi_r
                last = gi_mms(GIN, 2, last, True, True)          # gi_n
                last = bias_mm(z_reg, 3, last)
                last = w_mms(z_reg, 4, last, False)              # W_z
                last = gi_mms(z_reg, 1, last, False, True)       # gi_z

                # --- ACT chain ---
                r_sb = work.tile([128, R], f16, tag="r_sb")
                nc.scalar.activation(out=r_sb, in_=r_reg, func=Sigmoid)

                # stage GIN to SBUF fp16 (off critical path)
                gin_sb = work.tile([128, R], f16, tag="gin_sb")
                nc.vector.tensor_copy(gin_sb, GIN)

                # u_n = r * G_n ; pren = u_n + GIN   (fp16 SBUF, 2x mode)
                u_n = work.tile([128, R], f16, tag="u_n")
                nc.vector.tensor_mul(u_n, r_sb, gn_sb)
                nc.vector.tensor_add(PREN, u_n, gin_sb)

                n_sb = work.tile([128, R], f16, tag="n_sb")
                tanh_n_ins = nc.scalar.activation(out=n_sb, in_=PREN, func=Tanh)

                # zc = 1 - z = sigmoid(-G_z), directly on ACT (after tanh_n)
                zc = work.tile([128, R], f16, tag="zc")
                zc_ins = nc.scalar.activation(
                    out=zc, in_=z_reg, func=Sigmoid, scale=-1.0,
                )
                add_dep_helper(zc_ins.ins, tanh_n_ins.ins, sync=False)

                # h' = h + zc*(n - h)
                d_nh = work.tile([128, R], f16, tag="d_nh")
                nc.vector.tensor_sub(d_nh, n_sb, h_cur)
                e_up = work.tile([128, R], f16, tag="e_up")
                nc.vector.tensor_mul(e_up, zc, d_nh)
                nc.vector.tensor_add(h_new, h_cur, e_up)

                # next step's PH = bias + P.h_cur (now) + P.e (when e lands):
                # h_next = h_cur + e, and P distributes over the sum, so the
                # P.h_cur half runs in this step's idle PE window.
                last = bias_mm(ph_reg, 1, last)
                last = w_mms(ph_reg, 12, last, False)            # P.h_cur
                for j in range(KC):
                    for k in range(KC):
                        mm = nc.tensor.matmul(
                            ph_reg[:, j * bl:(j + 1) * bl],
                            w_block(12 + j, k),
                            e_up[:, k * bl:(k + 1) * bl],
                            start=False, stop=(k == KC - 1),
                            skip_group_check=True,
                        )
                        pe_order(mm, last)
                        last = mm

                # stream h' out:  outl[t, p, j, b]
                dst = d_out[bass.ds(t_dyn, 1)].rearrange("o p j b -> (o p) j b")
                nc.sync.dma_start(
                    out=dst, in_=h_new.rearrange("p (j b) -> p j b", b=bl)
                )
                return last

            # PH region for step 0: h=0, so bias alone
            nc.tensor.matmul(
                PHB[:, 0:R], BP, ES[:, 128:256],
                start=True, stop=True, skip_group_check=True,
            )

            n_blocks = t_steps // u_steps
            with tc.For_i(
                0, n_blocks, 1, hint_engines=(mybir.EngineType.PE,)
            ) as iv:
                S2 = io.tile([3, u_steps * bl], f16, tag="S2")
                MB = io.tile([1, u_steps * bl], f16, tag="MB")
                MBC = io.tile([128, u_steps * 4 * bl], f16, tag="MBC")
                nc.sync.dma_start(
                    out=MBC,
                    in_=d_mb[:, bass.ds(iv * (u_steps * 4 * bl), u_steps * 4 * bl)],
                )
                nc.sync.dma_start(
                    out=S2[1:2, :].rearrange("p (u b) -> p u b", b=bl),
                    in_=d_a[bass.ds(iv * u_steps, u_steps)].unsqueeze(0),
                )
                nc.sync.dma_start(out=S2[2:3, :], in_=d_one)
                nc.sync.dma_start(
                    out=MB[0:1, :].rearrange("p (u b) -> p u b", b=bl),
                    in_=d_m[bass.ds(iv * u_steps, u_steps)].unsqueeze(0),
                )
                prev_last = None
                for u in range(u_steps):
                    prev_last = emit_step(
                        u,
                        h_tiles[u % 2],
                        h_tiles[(u + 1) % 2],
                        S2,
                        MB,
                        iv * u_steps + u,
                        prev_last,
                    )

    nc.compile()
    return nc


def _prep_core_inputs(inputs, core, t_steps=T, bl=BL):
    """Build the per-core input map (numpy) for core id `core`."""
    f16 = np.float16
    direction = 0 if core < 4 else 1  # 0 fwd, 1 bwd
    bg = core % 4
    sl = slice(bg * bl, (bg + 1) * bl)

    x = np.asarray(inputs["x"], np.float32)[:, :, 0]      # [B, T]
    msk = np.asarray(inputs["mask"]).astype(np.float32)[:, :, 0]
    pfx = "wf" if direction == 0 else "wb"
    w_ih = np.asarray(inputs[f"{pfx}_ih"], np.float32)[:, 0]   # [3H]
    w_hh = np.asarray(inputs[f"{pfx}_hh"], np.float32)         # [3H, H]
    b_ih = np.asarray(inputs[f"b{pfx[1]}_ih"], np.float32)
    b_hh = np.asarray(inputs[f"b{pfx[1]}_hh"], np.float32)
    p_w1 = np.asarray(inputs["p_w1"], np.float32)
    p_b1 = np.asarray(inputs["p_b1"], np.float32)
    p_w2 = np.asarray(inputs["p_w2"], np.float32)
    p_b2 = np.asarray(inputs["p_b2"], np.float32)

    xs = x[sl].T.copy()      # [T, bl]
    ms = msk[sl].T.copy()
    if direction == 1:
        xs = xs[::-1].copy()
        ms = ms[::-1].copy()
    a_arr = (xs * (1.0 - ms)).astype(f16)
    m_arr = ms.astype(f16)

    W = np.concatenate([w_hh, p_w1], axis=0)             # [2048, 512]
    Wr = W.reshape(MC, 128, KC, 128)                     # [m, c, k, p]
    wt = Wr.transpose(3, 0, 2, 1).reshape(128, MC * KC * 128).astype(f16)

    # gi stationaries: per (gate g, chunk j) a [3,128] block:
    # rows 0,1 = w_ih chunk (contract with [pred*mask; a] rows of S2),
    # row  2   = bias chunk for GIN only (r/z biases ride the bias matmuls).
    gil = np.zeros((3, 12 * 128), np.float32)
    for g in range(3):
        for j in range(KC):
            blk = slice((g * KC + j) * 128, (g * KC + j + 1) * 128)
            wchunk = w_ih[g * H + j * 128: g * H + (j + 1) * 128]
            gil[0, blk] = wchunk
            gil[1, blk] = wchunk
            if g == 2:
                gil[2, blk] = b_ih[2 * H + j * 128: 2 * H + (j + 1) * 128]
    gil = gil.astype(f16)

    # shared bias stationary: rows 0-3 b_hh_n, 4-7 p_b1, 8-11 bias_r,
    # 12-15 bias_z; esel[:, ri*128:(ri+1)*128] selects region ri's rows.
    bias_r = b_ih[0:H] + b_hh[0:H]
    bias_z = b_ih[H:2 * H] + b_hh[H:2 * H]
    bpad = np.zeros((128, 128), np.float32)
    for j in range(KC):
        bpad[j, :] = b_hh[2 * H + j * 128: 2 * H + (j + 1) * 128]
        bpad[4 + j, :] = p_b1[j * 128:(j + 1) * 128]
        bpad[8 + j, :] = bias_r[j * 128:(j + 1) * 128]
        bpad[12 + j, :] = bias_z[j * 128:(j + 1) * 128]
    bpad = bpad.astype(f16)

    esel = np.zeros((128, 4 * 128), np.float16)
    for ri in range(4):
        for j in range(KC):
            esel[4 * ri + j, ri * 128 + j * bl: ri * 128 + (j + 1) * bl] = 1.0

    pw2t = p_w2[0].reshape(KC, 128).T.astype(f16).copy()
    pb2h = p_b2.reshape(1, 1).astype(f16)
    # mask broadcast to [128 partitions, (t, j, b)] for the fused masked relu
    m_bcast = np.broadcast_to(
        np.repeat(m_arr[:t_steps], KC, axis=0).reshape(1, t_steps * KC * bl),
        (128, t_steps * KC * bl),
    ).copy()

    return {
        "wt": wt, "gilhs": gil, "bpad": bpad, "esel": esel,
        "pw2t": pw2t, "pb2h": pb2h, "m_bcast": m_bcast,
        "ones_row": np.ones((1, U_DEF * bl), f16),
        "a_arr": a_arr[:t_steps], "m_arr": m_arr[:t_steps],
    }


def _assemble(results, t_steps=T, bl=BL):
    """results: list of 8 per-core dicts with 'outl' [T,128,KC,bl] fp16."""
    out = np.zeros((B, t_steps, 2 * H), np.float32)
    for core in range(NCORES):
        direction = 0 if core < 4 else 1
        bg = core % 4
        arr = np.asarray(results[core]["outl"], np.float16).astype(np.float32)
        # [t, p, j, b] -> [b, t, j, p] -> [b, t, 512]
        arr = arr.transpose(3, 0, 2, 1).reshape(bl, t_steps, H)
        if direction == 1:
            arr = arr[:, ::-1]
        out[bg * bl:(bg + 1) * bl, :, direction * H:(direction + 1) * H] = arr
    return out


def kernel(**inputs):
    from concourse.bass_utils import run_bass_kernel_spmd

    key = (T, U_DEF, BL)
    if key not in _cache:
        _cache[key] = _build_program(T, U_DEF, BL)
    nc = _cache[key]

    in_maps = [_prep_core_inputs(inputs, c) for c in range(NCORES)]
    res = run_bass_kernel_spmd(
        nc, in_maps, core_ids=list(range(NCORES)), trace=False
    )
    return _assemble(res.results)
